# revision 4
# baseline (speedup 1.0000x reference)
"""DLRM forward on 8 TRN2 NeuronCores — self-contained Bass kernel.

Data-parallel over batch (512 samples/core), activations kept transposed
(features on partitions, samples on the free dim) so every weight loads as
lhsT in natural layout.

  - BatchNorm: each core recomputes the small ContrastiveNet over the FULL
    batch in bf16 for exact batch stats (cheaper + more robust than an
    all-reduce), and over its own shard in fp32r for an accurate x_embed.
    pb2 cancels inside BN so it is never added.
  - Embedding: indirect-DMA row gathers (128 rows / instruction), reduce
    over the 50-history axis, PE transpose to features-on-partitions.
  - Interaction: pairs (i,j) tiled as 4-row x w-col rectangles over the
    192x192 triangle (invalid i>j pairs get zero weights); per 128-pair
    chunk the two factor tiles are built with replicating (step-0) DRAM ->
    SBUF DMAs from a bf16 DRAM copy of the 192-feature concat, multiplied
    on VectorE into bf16 rhs chunks.
  - Top MLP: bf16 GEMM with K streamed in groups; per-group PSUM partials
    are accumulated into f32 SBUF; then two small bf16 GEMMs + sigmoid.
"""

import numpy as np
import ml_dtypes

import concourse.bass as bass
import concourse.mybir as mybir
import concourse.tile as tile
import concourse.bacc as bacc
from concourse.bass_utils import run_bass_kernel_spmd
from concourse.masks import make_identity

BF16 = ml_dtypes.bfloat16

B = 4096
NCORES = 8
BL = B // NCORES
HIST = 50
VOCAB = 100000
EMB = 64
DENSE = 256
PIN = 768
PHID = 256
POUT = 64
BOT0 = 512
BOT1 = 64
ALL = 192
NPAIR_TRI = ALL * (ALL + 1) // 2          # 18528
TOP0 = 1024
TOP1 = 512
BN_EPS = 1e-5
P = 128

BR = 4
N_RECT = sum(BR * (ALL - a) for a in range(0, ALL, BR))   # 18816
N_CHUNK = N_RECT // P                                     # 147
W_ROWS = N_RECT + BOT1                                    # 18880
GK = 12                                                   # chunks per group

f32 = mybir.dt.float32
f32r = mybir.dt.float32r
bf16 = mybir.dt.bfloat16
i32 = mybir.dt.int32


def _chunk_meta():
    """Rect pair order + per-chunk segments (a, b, w, p0).

    Column-groups: for a in 0,4,..,188: for j in a..191 -> 4 pairs
    (i=a..a+3) with i fastest.  Chunk c = colgroups [32c, 32c+32); within a
    chunk, partition p = 4*(colgroup offset) + (i - a).
    """
    cgs = [(a, j) for a in range(0, ALL, BR) for j in range(a, ALL)]
    chunks = []
    for c in range(N_CHUNK):
        grp = cgs[c * 32:(c + 1) * 32]
        segs = []
        s = 0
        while s < 32:
            a, j0 = grp[s]
            e = s
            while e + 1 < 32 and grp[e + 1] == (a, grp[e][1] + 1):
                e += 1
            segs.append((a, j0, e - s + 1, 4 * s))
            s = e + 1
        chunks.append(segs)
    maxrow = [max(max(b + w - 1, a + BR - 1) for (a, b, w, _) in segs)
              for segs in chunks]
    low = [c for c in range(N_CHUNK) if maxrow[c] < P]
    high = [c for c in range(N_CHUNK) if maxrow[c] >= P]
    return cgs, chunks, low + high


_CGS, _CHUNKS, _ORDER = _chunk_meta()


def _tri_idx(i, j):
    return i * ALL - (i * (i - 1)) // 2 + (j - i)


def _build_w_perm(tw1):
    w = np.zeros((W_ROWS, TOP0), dtype=np.float32)
    for o, c in enumerate(_ORDER):
        base = o * P
        for p in range(P):
            a, j = _CGS[c * 32 + p // 4]
            i = a + p % 4
            if i <= j:
                w[base + p] = tw1[_tri_idx(i, j)]
    w[N_RECT:] = tw1[NPAIR_TRI:]
    return w.astype(BF16)


_NC_CACHE = None

# bias-pack columns
BB1, BB2, PB1, PB2, TB1, TB2, TB3, GAM, BET, EPS = 0, 4, 5, 7, 8, 16, 20, 21, 22, 23


def _build_nc():
    nc = bacc.Bacc("TRN2", target_bir_lowering=False, debug=False,
                   num_devices=NCORES)

    xdT_d = nc.dram_tensor("xdT", [DENSE, BL], f32r, kind="ExternalInput")
    xeT_own_d = nc.dram_tensor("xeT_own", [PIN, BL], f32r, kind="ExternalInput")
    xeT_bf_d = nc.dram_tensor("xeT_bf", [PIN, B], bf16, kind="ExternalInput")
    idx_d = nc.dram_tensor("idx", [BL, HIST], i32, kind="ExternalInput")
    tab_d = nc.dram_tensor("tab", [VOCAB, EMB], f32, kind="ExternalInput")
    pw1r_d = nc.dram_tensor("pw1r", [PIN, PHID], f32r, kind="ExternalInput")
    pw2r_d = nc.dram_tensor("pw2r", [PHID, POUT], f32r, kind="ExternalInput")
    pw1b_d = nc.dram_tensor("pw1b", [PIN, PHID], bf16, kind="ExternalInput")
    pw2b_d = nc.dram_tensor("pw2b", [PHID, POUT], bf16, kind="ExternalInput")
    bw1_d = nc.dram_tensor("bw1", [DENSE, BOT0], f32r, kind="ExternalInput")
    bw2_d = nc.dram_tensor("bw2", [BOT0, BOT1], f32r, kind="ExternalInput")
    wperm_d = nc.dram_tensor("wperm", [W_ROWS, TOP0], bf16, kind="ExternalInput")
    tw2_d = nc.dram_tensor("tw2b", [TOP0, TOP1], bf16, kind="ExternalInput")
    tw3_d = nc.dram_tensor("tw3b", [TOP1, 1], bf16, kind="ExternalInput")
    bias_d = nc.dram_tensor("biasp", [P, 24], f32, kind="ExternalInput")

    out_d = nc.dram_tensor("y", [1, BL], f32, kind="ExternalOutput")
    xemb_d = nc.dram_tensor("xemb", [BL, POUT], f32, kind="ExternalOutput")

    Act = mybir.ActivationFunctionType
    Alu = mybir.AluOpType

    with tile.TileContext(nc) as tc:
        with tc.tile_pool(name="const", bufs=1) as const_p, \
             tc.tile_pool(name="gath", bufs=2) as gath_p, \
             tc.tile_pool(name="stageA", bufs=2) as stA_p, \
             tc.tile_pool(name="once", bufs=1) as once_p, \
             tc.tile_pool(name="keep", bufs=1) as keep_p, \
             tc.tile_pool(name="wstr", bufs=GK + 2) as w_p, \
             tc.tile_pool(name="fct", bufs=4) as f_p, \
             tc.tile_pool(name="intp", bufs=GK + 2) as int_p, \
             tc.tile_pool(name="ps", bufs=3, space="PSUM") as ps, \
             tc.tile_pool(name="gps", bufs=2, space="PSUM") as gps, \
             tc.tile_pool(name="zps", bufs=1, space="PSUM") as zps, \
             tc.tile_pool(name="dram", bufs=1, space="DRAM") as dram_p:

            bias_t = const_p.tile([P, 24], f32)
            nc.sync.dma_start(bias_t[:], bias_d[:])
            ident = const_p.tile([P, P], f32)
            make_identity(nc, ident[:])

            # ---- embedding gathers (GPSIMD) ----
            gath_tiles = []
            for bt in range(4):
                idx_t = const_p.tile([P, HIST], i32, tag=f"idx{bt}")
                nc.sync.dma_start(idx_t[:], idx_d[bt * P:(bt + 1) * P, :])
                g_t = gath_p.tile([P, HIST, EMB], f32, tag="g")
                for h in range(HIST):
                    nc.gpsimd.indirect_dma_start(
                        out=g_t[:, h, :],
                        out_offset=None,
                        in_=tab_d[:],
                        in_offset=bass.IndirectOffsetOnAxis(
                            ap=idx_t[:, h:h + 1], axis=0),
                    )
                gath_tiles.append(g_t)

            # ---- small-weight loads ----
            def load_kxn(dram, kdim, ndim, dtype, tag):
                t = const_p.tile([P, kdim // P, ndim], dtype, tag=tag)
                nc.sync.dma_start(t[:], dram[:].rearrange("(o p) n -> p o n", p=P))
                return t

            pw1r_t = load_kxn(pw1r_d, PIN, PHID, f32r, "pw1r")
            pw2r_t = load_kxn(pw2r_d, PHID, POUT, f32r, "pw2r")
            pw1b_t = load_kxn(pw1b_d, PIN, PHID, bf16, "pw1b")
            pw2b_t = load_kxn(pw2b_d, PHID, POUT, bf16, "pw2b")
            bw1_t = load_kxn(bw1_d, DENSE, BOT0, f32r, "bw1")
            bw2_t = load_kxn(bw2_d, BOT0, BOT1, f32r, "bw2")
            tw2_t = load_kxn(tw2_d, TOP0, TOP1, bf16, "tw2")
            tw3_t = load_kxn(tw3_d, TOP1, 1, bf16, "tw3")
            xdT_t = load_kxn(xdT_d, DENSE, BL, f32r, "xdT")
            xeT_own_t = load_kxn(xeT_own_d, PIN, BL, f32r, "xeT_own")

            # ---- bf16 full-batch stats pass ----
            ssum = keep_p.tile([POUT, NCORES], f32, tag="ssum")
            ssq = keep_p.tile([POUT, NCORES], f32, tag="ssq")
            for sh in range(NCORES):
                xe_t = stA_p.tile([P, PIN // P, BL], bf16, tag="xebf")
                nc.sync.dma_start(
                    xe_t[:],
                    xeT_bf_d[:, sh * BL:(sh + 1) * BL]
                    .rearrange("(o p) n -> p o n", p=P))
                hbf = []
                for m in range(PHID // P):
                    hps = ps.tile([P, BL], f32, space="PSUM", tag="ps")
                    for k in range(PIN // P):
                        nc.tensor.matmul(hps[:],
                                         lhsT=pw1b_t[:, k, m * P:(m + 1) * P],
                                         rhs=xe_t[:, k], start=(k == 0),
                                         stop=(k == PIN // P - 1))
                    hsb = stA_p.tile([P, BL], bf16, tag=f"hbf{m}")
                    nc.scalar.activation(hsb[:], hps[:], Act.Relu,
                                         bias=bias_t[:, PB1 + m:PB1 + m + 1])
                    hbf.append(hsb)
                zp = ps.tile([P, BL], f32, space="PSUM", tag="ps")
                for k in range(PHID // P):
                    nc.tensor.matmul(zp[:POUT], lhsT=pw2b_t[:, k], rhs=hbf[k][:],
                                     start=(k == 0), stop=(k == PHID // P - 1))
                scr1 = stA_p.tile([POUT, BL], f32, tag="scr1")
                nc.scalar.activation(scr1[:], zp[:POUT], Act.Copy,
                                     accum_out=ssum[:, sh:sh + 1])
                scr2 = stA_p.tile([POUT, BL], f32, tag="scr2")
                nc.scalar.activation(scr2[:], zp[:POUT], Act.Square,
                                     accum_out=ssq[:, sh:sh + 1])

            # ---- BN stats -> scale/shift (pb2 cancels) ----
            stat = keep_p.tile([POUT, 8], f32, tag="stat")
            MU, VAR, MU2, SD, RSD, SCL, SHF, TMP = range(8)
            nc.vector.tensor_reduce(out=stat[:, MU:MU + 1], in_=ssum[:],
                                    op=Alu.add, axis=mybir.AxisListType.X)
            nc.vector.tensor_scalar_mul(stat[:, MU:MU + 1], stat[:, MU:MU + 1],
                                        1.0 / B)
            nc.vector.tensor_reduce(out=stat[:, VAR:VAR + 1], in_=ssq[:],
                                    op=Alu.add, axis=mybir.AxisListType.X)
            nc.vector.tensor_scalar_mul(stat[:, VAR:VAR + 1],
                                        stat[:, VAR:VAR + 1], 1.0 / B)
            nc.vector.tensor_tensor(stat[:, MU2:MU2 + 1], stat[:, MU:MU + 1],
                                    stat[:, MU:MU + 1], Alu.mult)
            nc.vector.tensor_tensor(stat[:, VAR:VAR + 1], stat[:, VAR:VAR + 1],
                                    stat[:, MU2:MU2 + 1], Alu.subtract)
            nc.scalar.activation(stat[:, SD:SD + 1], stat[:, VAR:VAR + 1],
                                 Act.Sqrt, bias=bias_t[:POUT, EPS:EPS + 1])
            nc.vector.reciprocal(stat[:, RSD:RSD + 1], stat[:, SD:SD + 1])
            nc.vector.tensor_tensor(stat[:, SCL:SCL + 1], stat[:, RSD:RSD + 1],
                                    bias_t[:POUT, GAM:GAM + 1], Alu.mult)
            nc.vector.tensor_tensor(stat[:, TMP:TMP + 1], stat[:, MU:MU + 1],
                                    stat[:, SCL:SCL + 1], Alu.mult)
            nc.vector.tensor_tensor(stat[:, SHF:SHF + 1],
                                    bias_t[:POUT, BET:BET + 1],
                                    stat[:, TMP:TMP + 1], Alu.subtract)

            # ---- own-shard contrastive net (fp32r) ----
            hT = []
            for m in range(PHID // P):
                hps = ps.tile([P, BL], f32, space="PSUM", tag="ps")
                for k in range(PIN // P):
                    nc.tensor.matmul(hps[:],
                                     lhsT=pw1r_t[:, k, m * P:(m + 1) * P],
                                     rhs=xeT_own_t[:, k], start=(k == 0),
                                     stop=(k == PIN // P - 1))
                hr = once_p.tile([P, BL], f32r, tag=f"hT{m}")
                nc.scalar.activation(hr[:], hps[:], Act.Relu,
                                     bias=bias_t[:, PB1 + m:PB1 + m + 1])
                hT.append(hr)
            zown = zps.tile([P, BL], f32, space="PSUM", tag="zown")
            for k in range(PHID // P):
                nc.tensor.matmul(zown[:POUT], lhsT=pw2r_t[:, k], rhs=hT[k][:],
                                 start=(k == 0), stop=(k == PHID // P - 1))
            xembT = keep_p.tile([POUT, BL], f32, tag="xembT")
            nc.vector.tensor_scalar(xembT[:], zown[:POUT],
                                    stat[:, SCL:SCL + 1], stat[:, SHF:SHF + 1],
                                    Alu.mult, Alu.add)
            xembT_bf = keep_p.tile([POUT, BL], bf16, tag="xembT_bf")
            nc.vector.tensor_copy(xembT_bf[:], xembT[:])
            for s in range(BL // P):
                tps = ps.tile([P, BL], f32, space="PSUM", tag="ps")
                nc.tensor.transpose(tps[:, :POUT], xembT[:, s * P:(s + 1) * P],
                                    ident[:POUT, :POUT])
                tsb = stA_p.tile([P, POUT], f32, tag="xe_out")
                nc.scalar.copy(tsb[:], tps[:, :POUT])
                nc.sync.dma_start(xemb_d[s * P:(s + 1) * P, :], tsb[:])

            # ---- bottom MLP (fp32r) ----
            b1T = []
            for m in range(BOT0 // P):
                bps = ps.tile([P, BL], f32, space="PSUM", tag="ps")
                for k in range(DENSE // P):
                    nc.tensor.matmul(bps[:],
                                     lhsT=bw1_t[:, k, m * P:(m + 1) * P],
                                     rhs=xdT_t[:, k], start=(k == 0),
                                     stop=(k == DENSE // P - 1))
                br_ = once_p.tile([P, BL], f32r, tag=f"b1T{m}")
                nc.scalar.activation(br_[:], bps[:], Act.Relu,
                                     bias=bias_t[:, BB1 + m:BB1 + m + 1])
                b1T.append(br_)
            b2ps = ps.tile([P, BL], f32, space="PSUM", tag="ps")
            for k in range(BOT0 // P):
                nc.tensor.matmul(b2ps[:BOT1], lhsT=bw2_t[:, k], rhs=b1T[k][:],
                                 start=(k == 0), stop=(k == BOT0 // P - 1))
            botT_bf = keep_p.tile([BOT1, BL], bf16, tag="botT_bf")
            nc.scalar.activation(botT_bf[:], b2ps[:BOT1], Act.Relu,
                                 bias=bias_t[:BOT1, BB2:BB2 + 1])

            # ---- embedding reduce + transpose ----
            embT_bf = keep_p.tile([EMB, BL], bf16, tag="embT_bf")
            for bt in range(4):
                esum = stA_p.tile([P, EMB], f32, tag="esum")
                nc.vector.tensor_reduce(
                    out=esum[:],
                    in_=gath_tiles[bt][:].rearrange("p h e -> p e h"),
                    op=Alu.add, axis=mybir.AxisListType.X)
                eps_t = ps.tile([P, BL], f32, space="PSUM", tag="ps")
                nc.tensor.transpose(eps_t[:EMB, :P], esum[:], ident[:])
                nc.scalar.copy(embT_bf[:, bt * P:(bt + 1) * P], eps_t[:EMB, :P])

            # ---- CT -> DRAM (lo rows 0..127, hi rows 128..191) ----
            ct_lo = dram_p.tile([P, BL], bf16)
            ct_hi = dram_p.tile([EMB, BL], bf16)
            nc.sync.dma_start(ct_lo[:BOT1, :], botT_bf[:])
            nc.sync.dma_start(ct_lo[BOT1:, :], xembT_bf[:])
            nc.sync.dma_start(ct_hi[:], embT_bf[:])

            def ct_src(row0):
                if row0 < P:
                    return ct_lo[:].tensor, row0 * BL
                return ct_hi[:].tensor, (row0 - P) * BL

            # ---- main GEMM ----
            seq = [("bot", None)] + [("int", c) for c in _ORDER]
            groups = [seq[i:i + GK] for i in range(0, len(seq), GK)]
            t1acc = [keep_p.tile([P, BL], f32, tag=f"t1acc{m}", name=f"t1acc{m}")
                     for m in range(TOP0 // P)]

            for gi, grp in enumerate(groups):
                rhs_tiles = []
                w_tiles = []
                for li, (kind, c) in enumerate(grp):
                    pos = gi * GK + li
                    wt = w_p.tile([P, TOP0], bf16, tag="wt")
                    if kind == "bot":
                        nc.scalar.dma_start(wt[:BOT1, :], wperm_d[N_RECT:, :])
                        w_tiles.append(wt)
                        rhs_tiles.append((botT_bf, BOT1))
                        continue
                    oi = pos - 1          # ordered interaction index
                    nc.scalar.dma_start(wt[:], wperm_d[oi * P:(oi + 1) * P, :])
                    w_tiles.append(wt)

                    f1 = f_p.tile([P, BL], bf16, tag="f1")
                    f2 = f_p.tile([P, BL], bf16, tag="f2")
                    for (a, b, w, p0) in _CHUNKS[c]:
                        t_, off = ct_src(a)
                        nc.sync.dma_start(
                            f1[p0:p0 + 4 * w, :],
                            bass.AP(t_, off, [[0, w], [BL, 4], [1, BL]]))
                        pieces = []
                        if b < P:
                            w1 = min(w, P - b)
                            pieces.append((b, w1, p0))
                            if w1 < w:
                                pieces.append((P, w - w1, p0 + 4 * w1))
                        else:
                            pieces.append((b, w, p0))
                        for (bb, ww, pp) in pieces:
                            t_, off = ct_src(bb)
                            nc.sync.dma_start(
                                f2[pp:pp + 4 * ww, :],
                                bass.AP(t_, off, [[BL, ww], [0, 4], [1, BL]]))
                    it = int_p.tile([P, BL], bf16, tag="intT")
                    nc.vector.tensor_tensor(it[:], f1[:], f2[:], Alu.mult)
                    rhs_tiles.append((it, P))

                for m in range(TOP0 // P):
                    gp = gps.tile([P, BL], f32, space="PSUM", tag="gps")
                    for li in range(len(grp)):
                        rhs, nk = rhs_tiles[li]
                        nc.tensor.matmul(
                            gp[:],
                            lhsT=w_tiles[li][:nk, m * P:(m + 1) * P],
                            rhs=rhs[:nk],
                            start=(li == 0), stop=(li == len(grp) - 1))
                    if gi == 0:
                        nc.scalar.copy(t1acc[m][:], gp[:])
                    else:
                        nc.vector.tensor_tensor(t1acc[m][:], t1acc[m][:],
                                                gp[:], Alu.add)

            # ---- top MLP tail ----
            t1T = []
            for m in range(TOP0 // P):
                tb = once_p.tile([P, BL], bf16, tag=f"t1T{m}")
                nc.scalar.activation(tb[:], t1acc[m][:], Act.Relu,
                                     bias=bias_t[:, TB1 + m:TB1 + m + 1])
                t1T.append(tb)
            t2T = []
            for m in range(TOP1 // P):
                tps2 = ps.tile([P, BL], f32, space="PSUM", tag="ps")
                for k in range(TOP0 // P):
                    nc.tensor.matmul(tps2[:],
                                     lhsT=tw2_t[:, k, m * P:(m + 1) * P],
                                     rhs=t1T[k][:], start=(k == 0),
                                     stop=(k == TOP0 // P - 1))
                tb2 = once_p.tile([P, BL], bf16, tag=f"t2T{m}")
                nc.scalar.activation(tb2[:], tps2[:], Act.Relu,
                                     bias=bias_t[:, TB2 + m:TB2 + m + 1])
                t2T.append(tb2)
            t3ps = ps.tile([P, BL], f32, space="PSUM", tag="ps")
            for k in range(TOP1 // P):
                nc.tensor.matmul(t3ps[:1], lhsT=tw3_t[:, k], rhs=t2T[k][:],
                                 start=(k == 0), stop=(k == TOP1 // P - 1))
            outT = once_p.tile([1, BL], f32, tag="outT")
            nc.scalar.activation(outT[:], t3ps[:1], Act.Sigmoid,
                                 bias=bias_t[:1, TB3:TB3 + 1])
            nc.sync.dma_start(out_d[:], outT[:])

    nc.compile()
    return nc


def _host_prep(inputs):
    x_sparse = np.asarray(inputs["x_sparse"]).astype(np.int32)
    x_dense = np.asarray(inputs["x_dense"], dtype=np.float32)
    x_ebp = np.asarray(inputs["x_embed_before_projection"], dtype=np.float32)
    tab = np.ascontiguousarray(np.asarray(inputs["emb_table"], dtype=np.float32))

    xeT = np.ascontiguousarray(x_ebp.T)
    xdT = np.ascontiguousarray(x_dense.T)
    wperm = _build_w_perm(np.asarray(inputs["tw1"], dtype=np.float32))

    bias = np.zeros((P, 24), dtype=np.float32)
    bias[:, EPS] = BN_EPS
    bb1 = np.asarray(inputs["bb1"], dtype=np.float32)
    for m in range(4):
        bias[:, BB1 + m] = bb1[m * P:(m + 1) * P]
    bias[:BOT1, BB2] = np.asarray(inputs["bb2"], dtype=np.float32)
    pb1 = np.asarray(inputs["pb1"], dtype=np.float32)
    for m in range(2):
        bias[:, PB1 + m] = pb1[m * P:(m + 1) * P]
    bias[:POUT, PB2] = np.asarray(inputs["pb2"], dtype=np.float32)
    tb1 = np.asarray(inputs["tb1"], dtype=np.float32)
    for m in range(8):
        bias[:, TB1 + m] = tb1[m * P:(m + 1) * P]
    tb2 = np.asarray(inputs["tb2"], dtype=np.float32)
    for m in range(4):
        bias[:, TB2 + m] = tb2[m * P:(m + 1) * P]
    bias[0, TB3] = np.asarray(inputs["tb3"], dtype=np.float32)[0]
    bias[:POUT, GAM] = np.asarray(inputs["bn_gamma"], dtype=np.float32)
    bias[:POUT, BET] = np.asarray(inputs["bn_beta"], dtype=np.float32)

    shared = {
        "xeT_bf": xeT.astype(BF16),
        "tab": tab,
        "pw1r": np.asarray(inputs["pw1"], dtype=np.float32),
        "pw2r": np.asarray(inputs["pw2"], dtype=np.float32),
        "pw1b": np.asarray(inputs["pw1"], dtype=np.float32).astype(BF16),
        "pw2b": np.asarray(inputs["pw2"], dtype=np.float32).astype(BF16),
        "bw1": np.asarray(inputs["bw1"], dtype=np.float32),
        "bw2": np.asarray(inputs["bw2"], dtype=np.float32),
        "wperm": wperm,
        "tw2b": np.asarray(inputs["tw2"], dtype=np.float32).astype(BF16),
        "tw3b": np.asarray(inputs["tw3"], dtype=np.float32).astype(BF16),
        "biasp": bias,
    }
    in_maps = []
    for c in range(NCORES):
        s = slice(c * BL, (c + 1) * BL)
        m = dict(shared)
        m["xdT"] = np.ascontiguousarray(xdT[:, s])
        m["xeT_own"] = np.ascontiguousarray(xeT[:, s])
        m["idx"] = np.ascontiguousarray(x_sparse[s])
        in_maps.append(m)
    return in_maps


def run(inputs, **kw):
    global _NC_CACHE
    if _NC_CACHE is None:
        _NC_CACHE = _build_nc()
    in_maps = _host_prep(inputs)
    res = run_bass_kernel_spmd(_NC_CACHE, in_maps,
                               core_ids=list(range(NCORES)), **kw)
    out = np.concatenate([r["y"].reshape(BL, 1) for r in res.results], axis=0)
    xemb = np.concatenate([r["xemb"] for r in res.results], axis=0)
    return (out.astype(np.float32), xemb.astype(np.float32)), res


def kernel(**inputs):
    outs, _ = run(inputs)
    return outs
